# revision 4
# baseline (speedup 1.0000x reference)
"""GQA decode-step attention layer (B=16, T=1, E=2048, QH=16, KVH=8, HD=128,
S=4096) on 8 TRN2 NeuronCores.

Sharding: tensor-parallel over heads. Core c owns KV head c and q heads
2c, 2c+1. Weights are head-sliced, the KV cache is head-sliced, and each
core produces a partial output projection; the host sums the 8 partials.
No device collective is needed.

Host-side prep per core (all part of the shard/relayout step):
  - all matmul operands cast to bf16 (validated: output rel err ~4e-3)
  - K cache shard relayout to d-major [B, 128, S] so score matmuls need no
    on-device transpose
  - V cache shard relayout to SBUF-tile-major [B, 128, 32*129] with a fused
    ones-column per 128-row tile (the ones column accumulates the softmax
    denominator during the probs@V matmul)
  - additive key-validity mask (idx < pos) in the score-grid layout
  - RoPE sin/cos tables, RMSNorm scale vectors (q pre-scaled by HD^-0.5)

Device per core: QKV projection -> RMSNorm + RoPE -> PE-transpose q/k ->
scores grid [128, 2*32] per batch (K^T tiles stationary) -> +mask, exp
(no max subtraction; |scores| <= ~6) -> probs@V_aug accumulation [2, 129]
-> new-token term folded in via a diag-masked exp(k_new^T q) selector
matmul against all batches' v rows -> divide by denominator -> transpose
-> partial output projection.

The updated cache rows equal the device-computed roped k / projected v;
the host scatters them into a copy of the caches (the attention already
accounts for the new token, old cache row pos is masked out).
"""

import numpy as np

B = 16
S = 4096
E = 2048
QH = 16
KVH = 8
HD = 128
G = QH // KVH          # q heads per core
NT = S // 128          # 32 s-tiles
ET = E // 128          # 16 e-tiles
VW = HD + 1            # v tile width incl. ones column
NEG = -1.0e9
NCORES = 8

_STATE = None


def _build():
    from concourse import bass, bacc, tile, mybir

    f32 = mybir.dt.float32
    bf16 = mybir.dt.bfloat16
    Exp = mybir.ActivationFunctionType.Exp
    Sqrt = mybir.ActivationFunctionType.Sqrt
    X = mybir.AxisListType.X
    mult = mybir.AluOpType.mult

    nc = bacc.Bacc("TRN2", target_bir_lowering=False, debug=False,
                   num_devices=NCORES)

    def inp(name, shape, dt):
        return nc.declare_dram_parameter(name, list(shape), dt, isOutput=False)

    def outp(name, shape, dt):
        return nc.declare_dram_parameter(name, list(shape), dt, isOutput=True)

    xT_d = inp("xT", [128, ET * B], bf16)          # inputs^T, e-tile-major
    wq_d = inp("wq", [128, ET * G * HD], bf16)     # Wq shard, e-tile-major
    wk_d = inp("wk", [128, ET * HD], bf16)
    wv_d = inp("wv", [128, ET * HD], bf16)
    wo_d = inp("wo", [128, G * E], bf16)           # [d, (h, e)]
    kt_d = inp("kt", [B, 128, S], bf16)            # K^T per batch
    vv_d = inp("vv", [B, 128, NT * VW], bf16)      # V tile-major + ones col
    mask_d = inp("mask", [B, 128, 2 * NT], f32)    # additive validity mask
    dmask_d = inp("dmask", [B, 2 * B], f32)        # diag selector mask
    cosf_d = inp("cosf", [B, G * HD], f32)
    sinf_d = inp("sinf", [B, G * HD], f32)
    qsc_d = inp("qsc", [B, G * HD], f32)           # q_scale * HD^-0.5, tiled
    ksc_d = inp("ksc", [B, HD], f32)
    id_d = inp("ident", [128, 128], f32)
    outp_d = outp("out_p", [B, E], f32)
    knew_d = outp("k_new", [B, HD], f32)
    vnew_d = outp("v_new", [B, HD], f32)

    with tile.TileContext(nc) as tc:
        with tc.tile_pool(name="w", bufs=1) as wp, \
             tc.tile_pool(name="stream", bufs=3) as sp, \
             tc.tile_pool(name="sm", bufs=4) as mp, \
             tc.tile_pool(name="ps", bufs=2, space=bass.MemorySpace.PSUM) as pp:

            # ---- persistent loads -------------------------------------
            XT = wp.tile([128, ET * B], bf16)
            WQ = wp.tile([128, ET * G * HD], bf16)
            WK = wp.tile([128, ET * HD], bf16)
            WV = wp.tile([128, ET * HD], bf16)
            WO = wp.tile([128, G * E], bf16)
            COSF = wp.tile([B, G * HD], f32)
            SINF = wp.tile([B, G * HD], f32)
            QSC = wp.tile([B, G * HD], f32)
            KSC = wp.tile([B, HD], f32)
            IDENT = wp.tile([128, 128], f32)
            DMASK = wp.tile([B, 2 * B], f32)
            nc.gpsimd.dma_start(out=XT[:], in_=xT_d[:])
            nc.gpsimd.dma_start(out=WQ[:], in_=wq_d[:])
            nc.gpsimd.dma_start(out=WK[:], in_=wk_d[:])
            nc.gpsimd.dma_start(out=WV[:], in_=wv_d[:])
            nc.gpsimd.dma_start(out=WO[:], in_=wo_d[:])
            nc.gpsimd.dma_start(out=COSF[:], in_=cosf_d[:])
            nc.gpsimd.dma_start(out=SINF[:], in_=sinf_d[:])
            nc.gpsimd.dma_start(out=QSC[:], in_=qsc_d[:])
            nc.gpsimd.dma_start(out=KSC[:], in_=ksc_d[:])
            nc.gpsimd.dma_start(out=IDENT[:], in_=id_d[:])
            nc.gpsimd.dma_start(out=DMASK[:], in_=dmask_d[:])
            EPS_T = wp.tile([B, 1], f32)
            nc.vector.memset(EPS_T[:], 1e-6)

            # ---- stage 1: QKV projection ------------------------------
            QW = G * HD  # 256
            q_ps = pp.tile([B, QW], f32, tag="psA")
            k_ps = pp.tile([B, HD], f32, tag="psB")
            v_ps = pp.tile([B, HD], f32, tag="psB")
            for t in range(ET):
                lt = XT[:, t * B:(t + 1) * B]
                nc.tensor.matmul(q_ps[:], lt, WQ[:, t * QW:(t + 1) * QW],
                                 start=(t == 0), stop=(t == ET - 1))
                nc.tensor.matmul(k_ps[:], lt, WK[:, t * HD:(t + 1) * HD],
                                 start=(t == 0), stop=(t == ET - 1))
                nc.tensor.matmul(v_ps[:], lt, WV[:, t * HD:(t + 1) * HD],
                                 start=(t == 0), stop=(t == ET - 1))

            # ---- stage 2: RMSNorm + RoPE ------------------------------
            def norm_rope(ps, width, nh, scale_sb, out_name):
                sq = mp.tile([B, width], f32, tag=out_name + "_sq")
                nc.scalar.square(sq[:], ps[:])
                var = mp.tile([B, nh], f32, tag=out_name + "_var")
                for h in range(nh):
                    nc.vector.reduce_sum(var[:, h:h + 1],
                                         sq[:, h * HD:(h + 1) * HD], axis=X)
                std = mp.tile([B, nh], f32, tag=out_name + "_std")
                nc.scalar.activation(std[:], var[:], Sqrt,
                                     bias=EPS_T[:], scale=1.0 / HD)
                rstd = mp.tile([B, nh], f32, tag=out_name + "_rstd")
                nc.vector.reciprocal(rstd[:], std[:])
                sn = mp.tile([B, width], f32, tag=out_name + "_sn")
                for h in range(nh):
                    nc.vector.tensor_scalar(sn[:, h * HD:(h + 1) * HD],
                                            ps[:, h * HD:(h + 1) * HD],
                                            rstd[:, h:h + 1], None, op0=mult)
                nc.vector.tensor_mul(sn[:], sn[:], scale_sb[:, :width])
                # rotate-half: sw[.., :64] = sn[.., 64:], sw[.., 64:] = sn[.., :64]
                sw = mp.tile([B, width], f32, tag=out_name + "_sw")
                half = HD // 2
                for h in range(nh):
                    nc.vector.tensor_copy(sw[:, h * HD:h * HD + half],
                                          sn[:, h * HD + half:(h + 1) * HD])
                    nc.vector.tensor_copy(sw[:, h * HD + half:(h + 1) * HD],
                                          sn[:, h * HD:h * HD + half])
                r1 = mp.tile([B, width], f32, tag=out_name + "_r1")
                nc.vector.tensor_mul(r1[:], sn[:], COSF[:, :width])
                r2 = mp.tile([B, width], f32, tag=out_name + "_r2")
                nc.vector.tensor_mul(r2[:], sw[:], SINF[:, :width])
                ro = mp.tile([B, width], f32, tag=out_name)
                nc.vector.tensor_add(ro[:], r1[:], r2[:])
                return ro

            qr = norm_rope(q_ps, QW, G, QSC, "qr")
            kr = norm_rope(k_ps, HD, 1, KSC, "kr")
            nc.sync.dma_start(out=knew_d[:], in_=kr[:])
            v_sb = mp.tile([B, HD], f32, tag="v_sb")
            nc.scalar.copy(v_sb[:], v_ps[:])
            nc.sync.dma_start(out=vnew_d[:], in_=v_sb[:])
            v_aug = wp.tile([B, VW], bf16)
            nc.vector.memset(v_aug[:, HD:VW], 1.0)
            nc.vector.tensor_copy(v_aug[:, 0:HD], v_ps[:])

            # ---- stage 3: transpose q, k to [d, .] --------------------
            qT = wp.tile([128, G * B], bf16)   # [d, (b, h)] col = 2b+h
            qTv = qT[:].rearrange("d (b h) -> d h b", h=G)
            for h in range(G):
                t_ps = pp.tile([128, B], f32, tag="psC")
                nc.tensor.transpose(t_ps[:], qr[:, h * HD:(h + 1) * HD],
                                    IDENT[:B, :B])
                nc.vector.tensor_copy(qTv[:, h, :], t_ps[:])
            kT = wp.tile([128, B], bf16)
            t_ps = pp.tile([128, B], f32, tag="psC")
            nc.tensor.transpose(t_ps[:], kr[:], IDENT[:B, :B])
            nc.vector.tensor_copy(kT[:], t_ps[:])

            # ---- stage 4: new-token scores + selector -----------------
            snew_ps = pp.tile([B, 2 * B], f32, tag="psC")
            nc.tensor.matmul(snew_ps[:], kT[:], qT[:], start=True, stop=True)
            sd = mp.tile([B, 2 * B], f32, tag="sd")
            nc.vector.tensor_add(sd[:], snew_ps[:], DMASK[:])
            p_sel = wp.tile([B, 2 * B], bf16)
            nc.scalar.activation(p_sel[:], sd[:], Exp)

            # ---- stage 5: attention over cache ------------------------
            xnT = wp.tile([128, G * B], bf16)  # [d, (b, h)], normalized
            for b in range(B):
                KT_b = sp.tile([128, S], bf16, tag="KT")
                nc.sync.dma_start(out=KT_b[:], in_=kt_d[b])
                V_b = sp.tile([128, NT * VW], bf16, tag="VV")
                nc.scalar.dma_start(out=V_b[:], in_=vv_d[b])
                M_b = sp.tile([128, 2 * NT], f32, tag="MASK")
                nc.gpsimd.dma_start(out=M_b[:], in_=mask_d[b])

                sc_ps = pp.tile([128, 2 * NT], f32, tag="psA")
                rq = qT[:, G * b:G * (b + 1)]
                for j in range(NT):
                    nc.tensor.matmul(sc_ps[:, 2 * j:2 * j + 2],
                                     KT_b[:, 128 * j:128 * (j + 1)], rq,
                                     start=(j == 0), stop=(j == NT - 1))
                smt = mp.tile([128, 2 * NT], f32, tag="smt")
                nc.vector.tensor_add(smt[:], sc_ps[:], M_b[:])
                p_b = mp.tile([128, 2 * NT], bf16, tag="p_b")
                nc.scalar.activation(p_b[:], smt[:], Exp)

                xa_ps = pp.tile([G, VW], f32, tag="psB")
                for j in range(NT):
                    nc.tensor.matmul(xa_ps[:], p_b[:, 2 * j:2 * j + 2],
                                     V_b[:, VW * j:VW * (j + 1)],
                                     start=(j == 0), stop=False)
                nc.tensor.matmul(xa_ps[:], p_sel[:, G * b:G * (b + 1)],
                                 v_aug[:], start=False, stop=True)

                linv = mp.tile([G, 1], f32, tag="linv")
                nc.vector.reciprocal(linv[:], xa_ps[:, HD:HD + 1])
                xn_b = mp.tile([G, HD], f32, tag="xn_b")
                nc.vector.tensor_scalar(xn_b[:], xa_ps[:, 0:HD],
                                        linv[:], None, op0=mult)
                xt_ps = pp.tile([128, G], f32, tag="psC")
                nc.tensor.transpose(xt_ps[:], xn_b[:], IDENT[:G, :G])
                nc.vector.tensor_copy(xnT[:, G * b:G * (b + 1)], xt_ps[:])

            # ---- stage 6: output projection ---------------------------
            xnTv = xnT[:].rearrange("d (b h) -> d h b", h=G)
            NE = 512
            for e in range(E // NE):
                o_ps = pp.tile([B, NE], f32, tag=("psA" if e % 2 == 0 else "psB"))
                for h in range(G):
                    nc.tensor.matmul(o_ps[:], xnTv[:, h, :],
                                     WO[:, h * E + e * NE:h * E + (e + 1) * NE],
                                     start=(h == 0), stop=(h == G - 1))
                o_sb = mp.tile([B, NE], f32, tag="o_sb")
                nc.scalar.copy(o_sb[:], o_ps[:])
                nc.sync.dma_start(out=outp_d[:, e * NE:(e + 1) * NE],
                                  in_=o_sb[:])

    nc.compile()
    return nc


def _prep_in_maps(inputs, positions, cache_key, cache_value, Wq, Wk, Wv, Wo,
                  q_scale, k_scale):
    import ml_dtypes

    bf16 = ml_dtypes.bfloat16
    x = np.ascontiguousarray(inputs.reshape(B, E))
    pos = positions.reshape(B).astype(np.int64)

    # shared (replicated) host-computed tables
    half = HD // 2
    freqs = (10000.0 ** (-np.arange(half, dtype=np.float64) / half))
    ang = pos[:, None].astype(np.float64) * freqs[None, :]
    cos = np.cos(ang).astype(np.float32)
    sin = np.sin(ang).astype(np.float32)
    cosf = np.tile(cos, (1, 2 * G)).astype(np.float32)              # [B, G*HD]
    sinf = np.tile(np.concatenate([-sin, sin], 1), (1, G)).astype(np.float32)
    qsc = (np.tile(q_scale, (B, G)) * HD ** -0.5).astype(np.float32)
    ksc = np.tile(k_scale, (B, 1)).astype(np.float32)
    ident = np.eye(128, dtype=np.float32)

    # inputs^T, e-tile-major [128, ET*B]
    xT = np.ascontiguousarray(
        x.T.reshape(ET, 128, B).transpose(1, 0, 2).reshape(128, ET * B)
    ).astype(bf16)

    # validity mask (idx < pos), score-grid layout [B, 128, 2*NT]
    sidx = np.arange(S).reshape(NT, 128).T                          # [128, NT]
    valid = sidx[None, :, :] < pos[:, None, None]                   # [B,128,NT]
    mask = np.where(np.repeat(valid, 2, axis=2), 0.0, NEG).astype(np.float32)

    dmask = np.full((B, 2 * B), NEG, dtype=np.float32)
    for b in range(B):
        dmask[b, 2 * b:2 * b + 2] = 0.0

    in_maps = []
    for c in range(NCORES):
        wq = Wq[:, G * c:G * (c + 1), :].reshape(E, G * HD)
        wq = wq.reshape(ET, 128, G * HD).transpose(1, 0, 2).reshape(128, -1)
        wk = Wk[:, c, :].reshape(ET, 128, HD).transpose(1, 0, 2).reshape(128, -1)
        wv = Wv[:, c, :].reshape(ET, 128, HD).transpose(1, 0, 2).reshape(128, -1)
        wo = Wo[G * c:G * (c + 1)].transpose(1, 0, 2).reshape(128, G * E)
        kt = cache_key[:, :, c, :].transpose(0, 2, 1)               # [B,128,S]
        vt = cache_value[:, :, c, :].reshape(B, NT, 128, HD).transpose(0, 2, 1, 3)
        vv = np.empty((B, 128, NT, VW), dtype=bf16)
        vv[..., :HD] = vt.astype(bf16)
        vv[..., HD] = bf16(1.0)
        in_maps.append({
            "xT": xT,
            "wq": np.ascontiguousarray(wq).astype(bf16),
            "wk": np.ascontiguousarray(wk).astype(bf16),
            "wv": np.ascontiguousarray(wv).astype(bf16),
            "wo": np.ascontiguousarray(wo).astype(bf16),
            "kt": np.ascontiguousarray(kt).astype(bf16),
            "vv": vv.reshape(B, 128, NT * VW),
            "mask": mask,
            "dmask": dmask,
            "cosf": cosf,
            "sinf": sinf,
            "qsc": qsc,
            "ksc": ksc,
            "ident": ident,
        })
    return in_maps


def _get_state():
    global _STATE
    if _STATE is None:
        _STATE = _build()
    return _STATE


def run_on_device(in_maps, **kw):
    from concourse.bass_utils import run_bass_kernel_spmd

    nc = _get_state()
    return run_bass_kernel_spmd(nc, in_maps, core_ids=list(range(NCORES)), **kw)


def kernel(inputs, positions, cache_key, cache_value, Wq, Wk, Wv, Wo,
           q_scale, k_scale, _bench_results=None):
    in_maps = _prep_in_maps(inputs, positions, cache_key, cache_value,
                            Wq, Wk, Wv, Wo, q_scale, k_scale)
    res = run_on_device(in_maps)
    if _bench_results is not None:
        _bench_results.append(res)
    results = res.results

    out = np.zeros((B, E), dtype=np.float32)
    for c in range(NCORES):
        out += results[c]["out_p"]

    pos = positions.reshape(B).astype(np.int64)
    new_key = np.array(cache_key, dtype=np.float32, copy=True)
    new_value = np.array(cache_value, dtype=np.float32, copy=True)
    bi = np.arange(B)
    for c in range(NCORES):
        new_key[bi, pos, c, :] = results[c]["k_new"]
        new_value[bi, pos, c, :] = results[c]["v_new"]

    return out.reshape(B, 1, E), new_key, new_value
